# revision 21
# baseline (speedup 1.0000x reference)
"""Trainium2 Bass kernel for a batch-first unrolled LSTM (nn_BaseRNN).

Reference computation (per batch element b, zero initial state):
    xg[t]   = x[t] @ Wx + b                      # [T, 4H], gate order (i, f, g, o)
    gates_t = xg[t] + h_{t-1} @ Wh
    i, f, g, o = split(gates_t)
    c_t = sigmoid(f) * c_{t-1} + sigmoid(i) * tanh(g)
    h_t = sigmoid(o) * tanh(c_t)
Returns (hs, cs), each [B, T, H].

Shapes: B=64, T=2048, D=H=128, 4H=512.  8 NeuronCores, data-parallel over
batch (B_LOC = 8 per core).

Design (per core) - v4, single-chain latency-optimized:
  * ONE dependency chain covering all 8 local batch elements ([128, 8]
    tiles, hidden dim on partitions).  A serial scan's wall-clock is
    T * (per-step critical-path latency); extra chains cannot reduce it but
    do add head-of-line blocking in the in-order engine queues, so we use
    exactly one (the previous 2-chain version measured 2.79us/step; this
    one 1.82us/step).
  * PSUM double buffering: bank set A (banks 0-3) holds the current chunk's
    per-gate accumulators, set B (banks 4-7) is pre-filled with xg for the
    next chunk while the current chunk scans.  TC = 64 steps/chunk
    (64 steps * 8 batch = 512 fp32 = one bank per gate).  A start=True
    matmul resets its whole PSUM bank, so each gate's xg is written by one
    full-width matmul (float32r moving operand: 1 cycle/row vs fp32's 4),
    spread at steps 2/18/34/50 to hide in PE idle time.
  * x is transposed on the HOST to [D, T, B] so xt tiles DMA directly as
    [128, TC*8] with 2KB contiguous lines - no PE transpose, no PSUM->SBUF
    copy on device.
  * Per step: 4 matmuls (h_{t-1} @ Wh_g accumulated onto xg in PSUM, fp16
    weights), ONE sigmoid over all 4 gates (g columns pre-scaled by 2 on
    host; tanh(g) = 2*sigmoid(2g) - 1 rebuilt with one fused DVE op), DVE
    chain gt/mt/at/c, one tanh, one output multiply feeding both the hh
    history tile and the next matmul's moving operand.
  * _optimize_sync post-pass on the Tile-scheduled stream:
      - drops same-engine semaphore self-waits on ACT and on the spaced
        DVE glue ops (in-order engines make them redundant; each costs
        producer drain + sem propagation ~90ns, and the sigmoid's second
        wait forced a per-step InstEventSemaphore helper);
      - reorders InstMatmult wait lists so the h-dependency stays on the
        matmul and the stale WAR wait moves to the InstLdweights, letting
        the first weight load execute during engine-idle time.
  * Outputs are written HBM-transposed ([H, T, B]) for full-bandwidth DMA;
    the host re-layouts to [B, T, H].
"""

import numpy as np
from contextlib import ExitStack

import concourse.bacc as bacc
import concourse.bass as bass
import concourse.mybir as mybir
import concourse.tile as tile
from concourse import bass_utils

F32 = mybir.dt.float32
F16 = mybir.dt.float16
F32R = mybir.dt.float32r
AF = mybir.ActivationFunctionType
OP = mybir.AluOpType

import os
PROBE = os.environ.get("PROBE", "")
OSPLIT = os.environ.get("OSPLIT", "0") == "1"
NDUMMY = int(os.environ.get("NDUMMY", "0"))
DUMMY_COLS = int(os.environ.get("DUMMY_COLS", "512"))

B_TOT, T_FULL, D, H = 64, 2048, 128, 128
G4 = 4 * H                      # 512
NCORES = 8
B_LOC = B_TOT // NCORES         # 8
TC = int(os.environ.get("TC", "64"))  # steps/chunk (<=64; bank=TC*B_LOC f32)
NPAR = 2
XG_PIECES = 1                   # sub-matmuls per gate (start=True resets whole bank)
ILEAVE = os.environ.get("ILEAVE", "0") == "1"  # interleaved psum gate layout


def build_lstm_nc(T: int = T_FULL, with_bias: bool = False) -> bacc.Bacc:
    nchunk = T // TC
    assert nchunk * TC == T and nchunk >= 2

    nc = bacc.Bacc("TRN2", target_bir_lowering=False, debug=False,
                   num_devices=NCORES)

    xt_d = nc.dram_tensor("xT", [D, T, B_LOC], F32R, kind="ExternalInput").ap()
    wx_d = nc.dram_tensor("wx", [D, G4], F32R, kind="ExternalInput").ap()
    wh_d = nc.dram_tensor("wh", [H, G4], F16, kind="ExternalInput").ap()
    if with_bias:
        b_d = nc.dram_tensor("bvec", [1, G4], F32, kind="ExternalInput").ap()
    hs_d = nc.dram_tensor("hsT", [H, T, B_LOC], F16, kind="ExternalOutput").ap()
    cs_d = nc.dram_tensor("csT", [H, T, B_LOC], F32, kind="ExternalOutput").ap()

    # Persistent SBUF
    wx_sb = nc.alloc_sbuf_tensor("wx_sb", [128, G4], F32R).ap()
    wh_sb = nc.alloc_sbuf_tensor("wh_sb", [128, G4], F16).ap()
    if with_bias:
        b_sb = nc.alloc_sbuf_tensor("b_sb", [1, G4], F32).ap()
        ones_sb = nc.alloc_sbuf_tensor("ones_sb", [1, G4], F32).ap()
    sg = [nc.alloc_sbuf_tensor(f"sg{p}", [128, 4 * B_LOC], F32).ap()
          for p in range(NPAR)]
    sgv = [s.rearrange("p (g q) -> p g q", g=4) for s in sg]
    gt = [nc.alloc_sbuf_tensor(f"gt{p}", [128, B_LOC], F32).ap()
          for p in range(NPAR)]
    at = [nc.alloc_sbuf_tensor(f"at{p}", [128, B_LOC], F32).ap()
          for p in range(NPAR)]
    mt = [nc.alloc_sbuf_tensor(f"mt{p}", [128, B_LOC], F32).ap()
          for p in range(NPAR)]
    th = [nc.alloc_sbuf_tensor(f"th{p}", [128, B_LOC], F32).ap()
          for p in range(NPAR)]
    h0 = nc.alloc_sbuf_tensor("h0", [128, B_LOC], F16).ap()
    c0 = nc.alloc_sbuf_tensor("c0", [128, B_LOC], F32).ap()
    if NDUMMY:
        zw = nc.alloc_sbuf_tensor("zw", [128, B_LOC], F32).ap()

    # All 8 PSUM banks as one tensor: set s in cols [s*2048, (s+1)*2048).
    # Plain layout: gate g at +g*512, step t at +t*8 (sigmoid gathers the 4
    # gates at stride 512).  Interleaved layout (ILEAVE): col = bank(t//16)*512
    # + (t%16)*32 + g*8 + b, so one step's 4 gates are 32 contiguous columns.
    ps = nc.alloc_psum_tensor("ps", [128, 8 * 512], F32).ap()
    psv = ps.rearrange("p (s g q) -> p s g q", s=2, g=4)
    psi = ps.rearrange("p (s bk t g q) -> p s bk t g q", s=2, bk=4, t=16, g=4)

    def mm_out(sset, g, t):
        if ILEAVE:
            o = sset * 2048 + (t // 16) * 512 + (t % 16) * 32 + g * 8
        else:
            o = sset * 2048 + g * 512 + t * B_LOC
        return ps[:, o:o + B_LOC]

    with tile.TileContext(nc) as tc_ctx, ExitStack() as ctx:
        xt_pool = ctx.enter_context(tc_ctx.tile_pool(name="xt", bufs=3))
        hh_pool = ctx.enter_context(tc_ctx.tile_pool(name="hh", bufs=3))
        ch_pool = ctx.enter_context(tc_ctx.tile_pool(name="ch", bufs=3))

        # ---- prologue: weights, state init, chunk-0 xg
        nc.sync.dma_start(wx_sb, wx_d)
        nc.sync.dma_start(wh_sb, wh_d)
        if with_bias:
            nc.sync.dma_start(b_sb, b_d)
            nc.gpsimd.memset(ones_sb, 1.0)
        nc.gpsimd.memset(h0, 0.0)
        nc.gpsimd.memset(c0, 0.0)
        if NDUMMY:
            nc.gpsimd.memset(zw, 0.0)

        def load_xt(k, xt_tile):
            nc.sync.dma_start(
                xt_tile[:, :].rearrange("p (t q) -> p t q", q=B_LOC),
                xt_d[:, k * TC:(k + 1) * TC, :])

        def xg_mm_il(sset, g, bk, xt_tile):
            out = psi[:, sset, bk, :, g, :]
            nc.tensor.matmul(
                out, wx_sb[:, g * 128:(g + 1) * 128],
                xt_tile[:, bk * 128:(bk + 1) * 128],
                start=(g == 0), stop=(g == 3) and not with_bias,
                skip_group_check=True)
            if with_bias:
                nc.tensor.matmul(
                    out, b_sb[:, g * 128:(g + 1) * 128],
                    ones_sb[:, bk * 128:(bk + 1) * 128],
                    start=False, stop=(g == 3), skip_group_check=True)

        def xg_mm(sset, g, c0_, c1_, xt_tile):
            if ILEAVE:
                for bk in range(4):
                    xg_mm_il(sset, g, bk, xt_tile)
                return
            nc.tensor.matmul(
                ps[:, sset * 2048 + g * 512 + c0_:sset * 2048 + g * 512 + c1_],
                wx_sb[:, g * 128:(g + 1) * 128],
                xt_tile[:, c0_:c1_],
                start=True, stop=not with_bias)
            if with_bias:
                nc.tensor.matmul(
                    ps[:, sset * 2048 + g * 512 + c0_:
                       sset * 2048 + g * 512 + c1_],
                    b_sb[:, g * 128:(g + 1) * 128],
                    ones_sb[:, c0_:c1_],
                    start=False, stop=True)

        xt_cur = xt_pool.tile([128, TC * B_LOC], F32R, tag="xt", name="xt_t")
        load_xt(0, xt_cur)
        for g in range(4):
            xg_mm(0, g, 0, TC * B_LOC, xt_cur)
        xt_next = xt_pool.tile([128, TC * B_LOC], F32R, tag="xt", name="xt_t")
        load_xt(1, xt_next)

        hh_prev = None
        ch_prev = None
        for k in range(nchunk):
            s = k % 2
            base = s * 2048
            hh = hh_pool.tile([128, TC * B_LOC], F16, tag="hh", name="hh_t")
            ch = ch_pool.tile([128, TC * B_LOC], F32, tag="ch", name="ch_t")

            # xg pieces for chunk k+1 (into the other bank set), one per
            # step starting at t=2: 4 gates x 8 pieces of 64 cols.
            pieces = []
            if k + 1 < nchunk:
                if ILEAVE:
                    for g in range(4):
                        for bk in range(4):
                            pieces.append((1 - s, g, bk, None))
                else:
                    for g in range(4):
                        for p_ in range(XG_PIECES):
                            w = TC * B_LOC // XG_PIECES
                            pieces.append((1 - s, g, p_ * w, (p_ + 1) * w))

            for t in range(TC):
                if k == 0 and t == 0:
                    hprev, cprev = h0, c0
                elif t == 0:
                    hprev = hh_prev[:, (TC - 1) * B_LOC:TC * B_LOC]
                    cprev = ch_prev[:, (TC - 1) * B_LOC:TC * B_LOC]
                else:
                    hprev = hh[:, (t - 1) * B_LOC:t * B_LOC]
                    cprev = ch[:, (t - 1) * B_LOC:t * B_LOC]
                P = t % NPAR
                sl = slice(t * B_LOC, (t + 1) * B_LOC)

                gate_order = (0, 1, 2, 3)
                if PROBE == "1mm":
                    gate_order = (0,)
                if PROBE != "nomm":
                    for g in gate_order:
                        if PROBE == "k64":
                            nc.tensor.matmul(
                                ps[:, base + g * 512 + t * B_LOC:
                                   base + g * 512 + (t + 1) * B_LOC],
                                wh_sb[0:64, g * 128:(g + 1) * 128],
                                hprev[0:64, :],
                                start=False, stop=False,
                                skip_group_check=True)
                        else:
                            nc.tensor.matmul(
                                mm_out(s, g, t),
                                wh_sb[:, g * 128:(g + 1) * 128],
                                hprev,
                                start=False, stop=False,
                                skip_group_check=True)

                # interleaved PE work for the next chunk
                step_gap = 4 if ILEAVE else 16
                if pieces and t >= 2 and (t - 2) % step_gap == 0 \
                        and (t - 2) // step_gap < len(pieces):
                    ss, g_, a_, b_ = pieces[(t - 2) // step_gap]
                    if ILEAVE:
                        xg_mm_il(ss, g_, a_, xt_next)
                    else:
                        xg_mm(ss, g_, a_, b_, xt_next)
                if t == 1 and k + 2 < nchunk:
                    t_ = xt_pool.tile([128, TC * B_LOC], F32R, tag="xt",
                                      name="xt_t")
                    load_xt(k + 2, t_)
                    xt_after = t_

                # sigmoid over all four gates (g pre-scaled by 2)
                if ILEAVE:
                    o_ = s * 2048 + (t // 16) * 512 + (t % 16) * 32
                    nc.scalar.activation(sg[P], ps[:, o_:o_ + 32], AF.Sigmoid)
                else:
                    nc.scalar.activation(sgv[P], psv[:, s, :, sl], AF.Sigmoid)
                # g~ = tanh(g) = 2*sig - 1
                nc.vector.tensor_scalar(
                    gt[P], sgv[P][:, 2, :], 2.0, -1.0, OP.mult, OP.add)
                nc.vector.tensor_tensor(mt[P], sgv[P][:, 1, :], cprev, OP.mult)
                nc.vector.tensor_tensor(at[P], sgv[P][:, 0, :], gt[P], OP.mult)
                nc.vector.tensor_tensor(ch[:, sl], at[P], mt[P], OP.add)
                do_dummy = NDUMMY > 0
                if PROBE == "notanh":
                    nc.vector.tensor_tensor(hh[:, sl], sgv[P][:, 3, :],
                                            ch[:, sl], OP.mult)
                else:
                    nc.scalar.activation(th[P], ch[:, sl], AF.Tanh)
                    nc.vector.tensor_tensor(hh[:, sl], sgv[P][:, 3, :], th[P],
                                            OP.mult)
                # PE pre-warm: a zero-effect matmul whose moving operand is
                # this step's c slice, so it launches when c_t completes and
                # keeps the PE busy until h arrives (weight loads then run at
                # the warm p-state instead of the idle clock).
                if do_dummy:
                    nc.tensor.matmul(
                        ps[0:B_LOC, (1 - s) * 2048:(1 - s) * 2048 + B_LOC],
                        zw, ch[:, sl],
                        start=False, stop=False, skip_group_check=True)

            # --- dump chunk outputs, rotate prefetch tiles
            nc.sync.dma_start(
                hs_d[:, k * TC:(k + 1) * TC, :],
                hh[:, :].rearrange("p (t q) -> p t q", q=B_LOC))
            nc.sync.dma_start(
                cs_d[:, k * TC:(k + 1) * TC, :],
                ch[:, :].rearrange("p (t q) -> p t q", q=B_LOC))
            hh_prev, ch_prev = hh, ch
            if k + 1 < nchunk:
                xt_cur = xt_next
                if k + 2 < nchunk:
                    xt_next = xt_after

    _optimize_sync(nc)
    nc.compile()
    return nc


def _optimize_sync(nc) -> None:
    """Pre-compile sync surgery on the Tile-scheduled instruction stream.

    1) Drop same-engine semaphore waits: each engine executes its queue in
       order, so a wait on the engine's own completion counter is always
       satisfied by issue order.  Tile emits them anyway, and they cost the
       producer's pipeline-drain + semaphore propagation (~90ns per hop on
       the serial scan's DVE chain).
    2) InstMatmult with multiple waits: put the DVE (h-ready) wait first.
       bacc's move_matmul_waits_to_ldweights keeps only the FIRST wait on
       the matmul and moves the rest to the preceding InstLdweights; with
       the stale WAR wait on the ldweights, the weight load executes during
       engine-idle time instead of serializing after h.
    """
    if os.environ.get("NOSYNCOPT", "0") == "1":
        return
    drop_engines = set(
        e for e in os.environ.get("DROPSELF", "Activation").split(",") if e)
    mmfix = os.environ.get("MMFIX", "1") == "1"
    dve_glue_drop = os.environ.get("DVEGLUE", "1") == "1"
    for bb in nc.m.functions[0].blocks:
        for inst in bb.instructions:
            si = inst.sync_info
            if si is None or not si.on_wait:
                continue
            nm = type(inst).__name__
            if nm == "InstEventSemaphore":
                continue
            eng = str(inst.engine).split(".")[-1]
            waits = list(si.on_wait)
            if eng in drop_engines and nm not in ("InstDMACopy",
                                                  "InstTriggeredCopy"):
                waits = [u for u in waits if not (
                    str(u.sync_type) == "semaphore"
                    and "ge-imm" in str(u.wait_mode)
                    and isinstance(u.ant_name, str)
                    and u.ant_name.startswith(eng + "_"))]
            if mmfix and nm == "InstMatmult" and len(waits) > 1:
                waits.sort(key=lambda u: 0 if (
                    isinstance(u.ant_name, str)
                    and u.ant_name.startswith("DVE")) else 1)
            if len(waits) != len(si.on_wait) or waits != list(si.on_wait):
                si.on_wait = waits
    if dve_glue_drop:
        _drop_spaced_dve_waits(nc)
    if os.environ.get("LDWHOIST", "0") == "1":
        _hoist_ldweights(nc)



def _retime_dummies(nc) -> None:
    """Give each PE warm-up dummy the same wait as the step's tanh
    (DVE >= c_t) so it launches as c completes instead of immediately,
    keeping the PE busy right up to h's arrival."""
    for bb in nc.m.functions[0].blocks:
        last_tanh_wait = None
        for inst in bb.instructions:
            nm = type(inst).__name__
            si = inst.sync_info
            if nm == "InstActivation" and si is not None and si.on_wait:
                w = [u for u in si.on_wait
                     if isinstance(u.ant_name, str)
                     and u.ant_name.startswith("DVE_")]
                if w:
                    last_tanh_wait = w[0]
            elif (nm == "InstMatmult" and last_tanh_wait is not None
                  and "dmov" in repr(inst.ins)):
                if si is None:
                    continue
                si.on_wait = [last_tanh_wait]


def _hoist_ldweights(nc) -> None:
    """Reorder each step's PE run [L1,M1,L2,M2,L3,M3,L4,M4] to
    [L1,L2,M1,L3,M2,L4,M3,M4]: max 2 loads outstanding (PE stationary
    double-buffer), one weight load moves off the post-h critical path."""
    for bb in nc.m.functions[0].blocks:
        insts = bb.instructions
        i = 0
        n = len(insts)
        while i + 7 < n:
            win = [insts[i + j] for j in range(8)]
            names = [type(x).__name__ for x in win]
            engs = [str(x.engine).endswith("PE") for x in win]
            if (names == ["InstLdweights", "InstMatmult"] * 4
                    and all(engs)
                    and all(getattr(win[j], "start_tensor_calc", True) is False
                            for j in (1, 3, 5, 7))):
                # [L1,M1,L2,M2,L3,M3,L4,M4] -> [L1,L2,M1,L3,M2,L4,M3,M4]
                order = [0, 2, 1, 4, 3, 6, 5, 7]
                new = [win[j] for j in order]
                for j in range(8):
                    insts[i + j] = new[j]
                i += 8
            else:
                i += 1


def _drop_spaced_dve_waits(nc) -> None:
    """Drop DVE self-waits on the mt and at glue ops.  Issue order is
    gt, mt, at, c, h: mt's producer (previous step's c) is >=2 instructions
    back and at's producer (gt) has mt in between, so the in-order DVE
    pipeline has already committed those writes; c reads at with no spacing
    so its wait stays."""
    for bb in nc.m.functions[0].blocks:
        dve = [i for i in bb.instructions
               if str(i.engine).endswith("DVE")]
        for idx, inst in enumerate(dve):
            if type(inst).__name__ != "InstTensorScalarPtr":
                continue
            for off in (1, 2):     # mt, at
                if idx + off >= len(dve):
                    continue
                tgt = dve[idx + off]
                if type(tgt).__name__ != "InstTensorTensor":
                    continue
                si = tgt.sync_info
                if si is None or not si.on_wait:
                    continue
                waits = [u for u in si.on_wait if not (
                    str(u.sync_type) == "semaphore"
                    and "ge-imm" in str(u.wait_mode)
                    and isinstance(u.ant_name, str)
                    and u.ant_name.startswith("DVE_"))]
                if len(waits) != len(si.on_wait):
                    si.on_wait = waits


_NC_CACHE: dict = {}


def _get_nc(T: int, with_bias: bool) -> bacc.Bacc:
    key = (T, with_bias)
    if key not in _NC_CACHE:
        _NC_CACHE[key] = build_lstm_nc(T, with_bias)
    return _NC_CACHE[key]


def prep_inputs(x, Wx, Wh, b):
    """Host-side prep: pre-scale the g-gate (tanh) columns by 2, transpose
    x to [D, T, B] per core."""
    wx_s = np.array(Wx, dtype=np.float32, copy=True)
    wh_s = np.array(Wh, dtype=np.float32, copy=True)
    b_s = np.array(b, dtype=np.float32, copy=True)
    wx_s[:, 2 * H:3 * H] *= 2.0
    wh_s[:, 2 * H:3 * H] *= 2.0
    b_s[2 * H:3 * H] *= 2.0
    with_bias = bool(np.any(b_s != 0.0))
    x = np.asarray(x, dtype=np.float32)
    wh_f16 = wh_s.astype(np.float16)
    in_maps = []
    for i in range(NCORES):
        m = {
            "xT": np.ascontiguousarray(
                x[i * B_LOC:(i + 1) * B_LOC].transpose(2, 1, 0)),
            "wx": wx_s,
            "wh": wh_f16,
        }
        if with_bias:
            m["bvec"] = b_s.reshape(1, G4)
        in_maps.append(m)
    return in_maps, with_bias


def run(x, Wx, Wh, b, T=None, trace=False):
    T = T if T is not None else x.shape[1]
    in_maps, with_bias = prep_inputs(x, Wx, Wh, b)
    nc = _get_nc(T, with_bias)
    res = bass_utils.run_bass_kernel_spmd(
        nc, in_maps, list(range(NCORES)), trace=trace)
    B = x.shape[0]
    hs = np.empty((B, T, H), dtype=np.float32)
    cs = np.empty((B, T, H), dtype=np.float32)
    for i in range(NCORES):
        hs[i * B_LOC:(i + 1) * B_LOC] = (
            res.results[i]["hsT"].astype(np.float32).transpose(2, 1, 0))
        cs[i * B_LOC:(i + 1) * B_LOC] = (
            res.results[i]["csT"].transpose(2, 1, 0))
    return (hs, cs), res


def kernel(x, Wx, Wh, b):
    (hs, cs), _ = run(x, Wx, Wh, b)
    return hs, cs


# revision 22
# speedup vs baseline: 1.1667x; 1.1667x over previous
"""Trainium2 Bass kernel for a batch-first unrolled LSTM (nn_BaseRNN).

Reference computation (per batch element b, zero initial state):
    xg[t]   = x[t] @ Wx + b                      # [T, 4H], gate order (i, f, g, o)
    gates_t = xg[t] + h_{t-1} @ Wh
    i, f, g, o = split(gates_t)
    c_t = sigmoid(f) * c_{t-1} + sigmoid(i) * tanh(g)
    h_t = sigmoid(o) * tanh(c_t)
Returns (hs, cs), each [B, T, H].

Shapes: B=64, T=2048, D=H=128, 4H=512.  8 NeuronCores, data-parallel over
batch (B_LOC = 8 per core).

Design (per core) - v4, single-chain latency-optimized:
  * ONE dependency chain covering all 8 local batch elements ([128, 8]
    tiles, hidden dim on partitions).  A serial scan's wall-clock is
    T * (per-step critical-path latency); extra chains cannot reduce it but
    do add head-of-line blocking in the in-order engine queues, so we use
    exactly one (the previous 2-chain version measured 2.79us/step; this
    one 1.82us/step).
  * PSUM double buffering: bank set A (banks 0-3) holds the current chunk's
    per-gate accumulators, set B (banks 4-7) is pre-filled with xg for the
    next chunk while the current chunk scans.  TC = 64 steps/chunk
    (64 steps * 8 batch = 512 fp32 = one bank per gate).  A start=True
    matmul resets its whole PSUM bank, so each gate's xg is written by one
    full-width matmul (float32r moving operand: 1 cycle/row vs fp32's 4),
    spread at steps 2/18/34/50 to hide in PE idle time.
  * x is transposed on the HOST to [D, T, B] so xt tiles DMA directly as
    [128, TC*8] with 2KB contiguous lines - no PE transpose, no PSUM->SBUF
    copy on device.
  * Per step: 4 matmuls (h_{t-1} @ Wh_g accumulated onto xg in PSUM, fp16
    weights), ONE sigmoid over all 4 gates (g columns pre-scaled by 2 on
    host; tanh(g) = 2*sigmoid(2g) - 1 rebuilt with one fused DVE op), DVE
    chain gt/mt/at/c, one tanh, one output multiply feeding both the hh
    history tile and the next matmul's moving operand.
  * _optimize_sync post-pass on the Tile-scheduled stream:
      - drops same-engine semaphore self-waits on ACT and on the spaced
        DVE glue ops (in-order engines make them redundant; each costs
        producer drain + sem propagation ~90ns, and the sigmoid's second
        wait forced a per-step InstEventSemaphore helper);
      - reorders InstMatmult wait lists so the h-dependency stays on the
        matmul and the stale WAR wait moves to the InstLdweights, letting
        the first weight load execute during engine-idle time.
  * Outputs are written HBM-transposed ([H, T, B]) for full-bandwidth DMA;
    the host re-layouts to [B, T, H].
"""

import numpy as np
from contextlib import ExitStack

import concourse.bacc as bacc
import concourse.bass as bass
import concourse.mybir as mybir
import concourse.tile as tile
from concourse import bass_utils

F32 = mybir.dt.float32
F16 = mybir.dt.float16
F32R = mybir.dt.float32r
AF = mybir.ActivationFunctionType
OP = mybir.AluOpType

import os
PROBE = os.environ.get("PROBE", "")
OSPLIT = os.environ.get("OSPLIT", "0") == "1"
NDUMMY = int(os.environ.get("NDUMMY", "0"))
DUMMY_COLS = int(os.environ.get("DUMMY_COLS", "512"))

B_TOT, T_FULL, D, H = 64, 2048, 128, 128
G4 = 4 * H                      # 512
NCORES = 8
B_LOC = B_TOT // NCORES         # 8
TC = int(os.environ.get("TC", "64"))  # steps/chunk (<=64; bank=TC*B_LOC f32)
NPAR = 2
XG_PIECES = 1                   # sub-matmuls per gate (start=True resets whole bank)
ILEAVE = os.environ.get("ILEAVE", "0") == "1"  # interleaved psum gate layout


def build_lstm_nc(T: int = T_FULL, with_bias: bool = False) -> bacc.Bacc:
    nchunk = T // TC
    assert nchunk * TC == T and nchunk >= 2

    nc = bacc.Bacc("TRN2", target_bir_lowering=False, debug=False,
                   num_devices=NCORES)

    xt_d = nc.dram_tensor("xT", [D, T, B_LOC], F32R, kind="ExternalInput").ap()
    wx_d = nc.dram_tensor("wx", [D, G4], F32R, kind="ExternalInput").ap()
    wh_d = nc.dram_tensor("wh", [H, G4], F16, kind="ExternalInput").ap()
    if with_bias:
        b_d = nc.dram_tensor("bvec", [1, G4], F32, kind="ExternalInput").ap()
    hs_d = nc.dram_tensor("hsT", [H, T, B_LOC], F16, kind="ExternalOutput").ap()
    cs_d = nc.dram_tensor("csT", [H, T, B_LOC], F32, kind="ExternalOutput").ap()

    # Persistent SBUF
    wx_sb = nc.alloc_sbuf_tensor("wx_sb", [128, G4], F32R).ap()
    wh_sb = nc.alloc_sbuf_tensor("wh_sb", [128, G4], F16).ap()
    if with_bias:
        b_sb = nc.alloc_sbuf_tensor("b_sb", [1, G4], F32).ap()
        ones_sb = nc.alloc_sbuf_tensor("ones_sb", [1, G4], F32).ap()
    sg = [nc.alloc_sbuf_tensor(f"sg{p}", [128, 4 * B_LOC], F32).ap()
          for p in range(NPAR)]
    sgv = [s.rearrange("p (g q) -> p g q", g=4) for s in sg]
    gt = [nc.alloc_sbuf_tensor(f"gt{p}", [128, B_LOC], F32).ap()
          for p in range(NPAR)]
    at = [nc.alloc_sbuf_tensor(f"at{p}", [128, B_LOC], F32).ap()
          for p in range(NPAR)]
    mt = [nc.alloc_sbuf_tensor(f"mt{p}", [128, B_LOC], F32).ap()
          for p in range(NPAR)]
    th = [nc.alloc_sbuf_tensor(f"th{p}", [128, B_LOC], F32).ap()
          for p in range(NPAR)]
    h0 = nc.alloc_sbuf_tensor("h0", [128, B_LOC], F16).ap()
    c0 = nc.alloc_sbuf_tensor("c0", [128, B_LOC], F32).ap()
    spc = nc.alloc_sbuf_tensor("spc", [128, B_LOC], F32).ap()
    if NDUMMY:
        zw = nc.alloc_sbuf_tensor("zw", [128, B_LOC], F32).ap()

    # All 8 PSUM banks as one tensor: set s in cols [s*2048, (s+1)*2048).
    # Plain layout: gate g at +g*512, step t at +t*8 (sigmoid gathers the 4
    # gates at stride 512).  Interleaved layout (ILEAVE): col = bank(t//16)*512
    # + (t%16)*32 + g*8 + b, so one step's 4 gates are 32 contiguous columns.
    ps = nc.alloc_psum_tensor("ps", [128, 8 * 512], F32).ap()
    psv = ps.rearrange("p (s g q) -> p s g q", s=2, g=4)
    psi = ps.rearrange("p (s bk t g q) -> p s bk t g q", s=2, bk=4, t=16, g=4)

    def mm_out(sset, g, t):
        if ILEAVE:
            o = sset * 2048 + (t // 16) * 512 + (t % 16) * 32 + g * 8
        else:
            o = sset * 2048 + g * 512 + t * B_LOC
        return ps[:, o:o + B_LOC]

    with tile.TileContext(nc) as tc_ctx, ExitStack() as ctx:
        xt_pool = ctx.enter_context(tc_ctx.tile_pool(name="xt", bufs=3))
        hh_pool = ctx.enter_context(tc_ctx.tile_pool(name="hh", bufs=3))
        ch_pool = ctx.enter_context(tc_ctx.tile_pool(name="ch", bufs=3))

        # ---- prologue: weights, state init, chunk-0 xg
        nc.sync.dma_start(wx_sb, wx_d)
        nc.sync.dma_start(wh_sb, wh_d)
        if with_bias:
            nc.sync.dma_start(b_sb, b_d)
            nc.gpsimd.memset(ones_sb, 1.0)
        nc.gpsimd.memset(h0, 0.0)
        nc.gpsimd.memset(c0, 0.0)
        if NDUMMY:
            nc.gpsimd.memset(zw, 0.0)

        def load_xt(k, xt_tile):
            nc.sync.dma_start(
                xt_tile[:, :].rearrange("p (t q) -> p t q", q=B_LOC),
                xt_d[:, k * TC:(k + 1) * TC, :])

        def xg_mm_il(sset, g, bk, xt_tile):
            out = psi[:, sset, bk, :, g, :]
            nc.tensor.matmul(
                out, wx_sb[:, g * 128:(g + 1) * 128],
                xt_tile[:, bk * 128:(bk + 1) * 128],
                start=(g == 0), stop=(g == 3) and not with_bias,
                skip_group_check=True)
            if with_bias:
                nc.tensor.matmul(
                    out, b_sb[:, g * 128:(g + 1) * 128],
                    ones_sb[:, bk * 128:(bk + 1) * 128],
                    start=False, stop=(g == 3), skip_group_check=True)

        def xg_mm(sset, g, c0_, c1_, xt_tile):
            if ILEAVE:
                for bk in range(4):
                    xg_mm_il(sset, g, bk, xt_tile)
                return
            nc.tensor.matmul(
                ps[:, sset * 2048 + g * 512 + c0_:sset * 2048 + g * 512 + c1_],
                wx_sb[:, g * 128:(g + 1) * 128],
                xt_tile[:, c0_:c1_],
                start=True, stop=not with_bias)
            if with_bias:
                nc.tensor.matmul(
                    ps[:, sset * 2048 + g * 512 + c0_:
                       sset * 2048 + g * 512 + c1_],
                    b_sb[:, g * 128:(g + 1) * 128],
                    ones_sb[:, c0_:c1_],
                    start=False, stop=True)

        xt_cur = xt_pool.tile([128, TC * B_LOC], F32R, tag="xt", name="xt_t")
        load_xt(0, xt_cur)
        for g in range(4):
            xg_mm(0, g, 0, TC * B_LOC, xt_cur)
        xt_next = xt_pool.tile([128, TC * B_LOC], F32R, tag="xt", name="xt_t")
        load_xt(1, xt_next)

        hh_prev = None
        ch_prev = None
        for k in range(nchunk):
            s = k % 2
            base = s * 2048
            hh = hh_pool.tile([128, TC * B_LOC], F16, tag="hh", name="hh_t")
            ch = ch_pool.tile([128, TC * B_LOC], F32, tag="ch", name="ch_t")

            # xg pieces for chunk k+1 (into the other bank set), one per
            # step starting at t=2: 4 gates x 8 pieces of 64 cols.
            pieces = []
            if k + 1 < nchunk:
                if ILEAVE:
                    for g in range(4):
                        for bk in range(4):
                            pieces.append((1 - s, g, bk, None))
                else:
                    for g in range(4):
                        for p_ in range(XG_PIECES):
                            w = TC * B_LOC // XG_PIECES
                            pieces.append((1 - s, g, p_ * w, (p_ + 1) * w))

            for t in range(TC):
                if k == 0 and t == 0:
                    hprev, cprev = h0, c0
                elif t == 0:
                    hprev = hh_prev[:, (TC - 1) * B_LOC:TC * B_LOC]
                    cprev = ch_prev[:, (TC - 1) * B_LOC:TC * B_LOC]
                else:
                    hprev = hh[:, (t - 1) * B_LOC:t * B_LOC]
                    cprev = ch[:, (t - 1) * B_LOC:t * B_LOC]
                P = t % NPAR
                sl = slice(t * B_LOC, (t + 1) * B_LOC)

                gate_order = (0, 1, 2, 3)
                if PROBE == "1mm":
                    gate_order = (0,)
                if PROBE != "nomm":
                    for g in gate_order:
                        if PROBE == "k64":
                            nc.tensor.matmul(
                                ps[:, base + g * 512 + t * B_LOC:
                                   base + g * 512 + (t + 1) * B_LOC],
                                wh_sb[0:64, g * 128:(g + 1) * 128],
                                hprev[0:64, :],
                                start=False, stop=False,
                                skip_group_check=True)
                        else:
                            nc.tensor.matmul(
                                mm_out(s, g, t),
                                wh_sb[:, g * 128:(g + 1) * 128],
                                hprev,
                                start=False, stop=False,
                                skip_group_check=True)

                # interleaved PE work for the next chunk
                step_gap = 4 if ILEAVE else 16
                if pieces and t >= 2 and (t - 2) % step_gap == 0 \
                        and (t - 2) // step_gap < len(pieces):
                    ss, g_, a_, b_ = pieces[(t - 2) // step_gap]
                    if ILEAVE:
                        xg_mm_il(ss, g_, a_, xt_next)
                    else:
                        xg_mm(ss, g_, a_, b_, xt_next)
                if t == 1 and k + 2 < nchunk:
                    t_ = xt_pool.tile([128, TC * B_LOC], F32R, tag="xt",
                                      name="xt_t")
                    load_xt(k + 2, t_)
                    xt_after = t_

                # sigmoid over all four gates (g pre-scaled by 2)
                if ILEAVE:
                    o_ = s * 2048 + (t // 16) * 512 + (t % 16) * 32
                    nc.scalar.activation(sg[P], ps[:, o_:o_ + 32], AF.Sigmoid)
                else:
                    nc.scalar.activation(sgv[P], psv[:, s, :, sl], AF.Sigmoid)
                # g~ = tanh(g) = 2*sig - 1
                nc.vector.tensor_scalar(
                    gt[P], sgv[P][:, 2, :], 2.0, -1.0, OP.mult, OP.add)
                nc.vector.tensor_tensor(mt[P], sgv[P][:, 1, :], cprev, OP.mult)
                nc.vector.tensor_tensor(at[P], sgv[P][:, 0, :], gt[P], OP.mult)
                # spacer: covers at's write-commit window so c's same-engine
                # wait can be dropped by _drop_spaced_dve_waits
                nc.vector.tensor_tensor(spc, gt[P], gt[P], OP.mult)
                nc.vector.tensor_tensor(ch[:, sl], at[P], mt[P], OP.add)
                do_dummy = NDUMMY > 0
                if PROBE == "notanh":
                    nc.vector.tensor_tensor(hh[:, sl], sgv[P][:, 3, :],
                                            ch[:, sl], OP.mult)
                else:
                    nc.scalar.activation(th[P], ch[:, sl], AF.Tanh)
                    nc.vector.tensor_tensor(hh[:, sl], sgv[P][:, 3, :], th[P],
                                            OP.mult)
                # PE pre-warm: a zero-effect matmul whose moving operand is
                # this step's c slice, so it launches when c_t completes and
                # keeps the PE busy until h arrives (weight loads then run at
                # the warm p-state instead of the idle clock).
                if do_dummy:
                    nc.tensor.matmul(
                        ps[0:B_LOC, (1 - s) * 2048:(1 - s) * 2048 + B_LOC],
                        zw, ch[:, sl],
                        start=False, stop=False, skip_group_check=True)

            # --- dump chunk outputs, rotate prefetch tiles
            nc.sync.dma_start(
                hs_d[:, k * TC:(k + 1) * TC, :],
                hh[:, :].rearrange("p (t q) -> p t q", q=B_LOC))
            nc.sync.dma_start(
                cs_d[:, k * TC:(k + 1) * TC, :],
                ch[:, :].rearrange("p (t q) -> p t q", q=B_LOC))
            hh_prev, ch_prev = hh, ch
            if k + 1 < nchunk:
                xt_cur = xt_next
                if k + 2 < nchunk:
                    xt_next = xt_after

    _optimize_sync(nc)
    nc.compile()
    return nc


def _optimize_sync(nc) -> None:
    """Pre-compile sync surgery on the Tile-scheduled instruction stream.

    1) Drop same-engine semaphore waits: each engine executes its queue in
       order, so a wait on the engine's own completion counter is always
       satisfied by issue order.  Tile emits them anyway, and they cost the
       producer's pipeline-drain + semaphore propagation (~90ns per hop on
       the serial scan's DVE chain).
    2) InstMatmult with multiple waits: put the DVE (h-ready) wait first.
       bacc's move_matmul_waits_to_ldweights keeps only the FIRST wait on
       the matmul and moves the rest to the preceding InstLdweights; with
       the stale WAR wait on the ldweights, the weight load executes during
       engine-idle time instead of serializing after h.
    """
    if os.environ.get("NOSYNCOPT", "0") == "1":
        return
    drop_engines = set(
        e for e in os.environ.get("DROPSELF", "Activation").split(",") if e)
    mmfix = os.environ.get("MMFIX", "1") == "1"
    dve_glue_drop = os.environ.get("DVEGLUE", "1") == "1"
    for bb in nc.m.functions[0].blocks:
        for inst in bb.instructions:
            si = inst.sync_info
            if si is None or not si.on_wait:
                continue
            nm = type(inst).__name__
            if nm == "InstEventSemaphore":
                continue
            eng = str(inst.engine).split(".")[-1]
            waits = list(si.on_wait)
            if eng in drop_engines and nm not in ("InstDMACopy",
                                                  "InstTriggeredCopy"):
                waits = [u for u in waits if not (
                    str(u.sync_type) == "semaphore"
                    and "ge-imm" in str(u.wait_mode)
                    and isinstance(u.ant_name, str)
                    and u.ant_name.startswith(eng + "_"))]
            if mmfix and nm == "InstMatmult" and len(waits) > 1:
                waits.sort(key=lambda u: 0 if (
                    isinstance(u.ant_name, str)
                    and u.ant_name.startswith("DVE")) else 1)
            if len(waits) != len(si.on_wait) or waits != list(si.on_wait):
                si.on_wait = waits
    if dve_glue_drop:
        _drop_spaced_dve_waits(nc)
    if os.environ.get("LDWHOIST", "0") == "1":
        _hoist_ldweights(nc)



def _retime_dummies(nc) -> None:
    """Give each PE warm-up dummy the same wait as the step's tanh
    (DVE >= c_t) so it launches as c completes instead of immediately,
    keeping the PE busy right up to h's arrival."""
    for bb in nc.m.functions[0].blocks:
        last_tanh_wait = None
        for inst in bb.instructions:
            nm = type(inst).__name__
            si = inst.sync_info
            if nm == "InstActivation" and si is not None and si.on_wait:
                w = [u for u in si.on_wait
                     if isinstance(u.ant_name, str)
                     and u.ant_name.startswith("DVE_")]
                if w:
                    last_tanh_wait = w[0]
            elif (nm == "InstMatmult" and last_tanh_wait is not None
                  and "dmov" in repr(inst.ins)):
                if si is None:
                    continue
                si.on_wait = [last_tanh_wait]


def _hoist_ldweights(nc) -> None:
    """Reorder each step's PE run [L1,M1,L2,M2,L3,M3,L4,M4] to
    [L1,L2,M1,L3,M2,L4,M3,M4]: max 2 loads outstanding (PE stationary
    double-buffer), one weight load moves off the post-h critical path."""
    for bb in nc.m.functions[0].blocks:
        insts = bb.instructions
        i = 0
        n = len(insts)
        while i + 7 < n:
            win = [insts[i + j] for j in range(8)]
            names = [type(x).__name__ for x in win]
            engs = [str(x.engine).endswith("PE") for x in win]
            if (names == ["InstLdweights", "InstMatmult"] * 4
                    and all(engs)
                    and all(getattr(win[j], "start_tensor_calc", True) is False
                            for j in (1, 3, 5, 7))):
                # [L1,M1,L2,M2,L3,M3,L4,M4] -> [L1,L2,M1,L3,M2,L4,M3,M4]
                order = [0, 2, 1, 4, 3, 6, 5, 7]
                new = [win[j] for j in order]
                for j in range(8):
                    insts[i + j] = new[j]
                i += 8
            else:
                i += 1


def _drop_spaced_dve_waits(nc) -> None:
    """Drop DVE self-waits on the mt and at glue ops.  Issue order is
    gt, mt, at, spacer, c, h: every consumer is >=2 issue slots after its
    producer (mt reads the previous step's c; at reads gt across mt; c reads
    at across the spacer), so the in-order DVE pipeline has already
    committed the writes and the semaphore waits are redundant."""
    for bb in nc.m.functions[0].blocks:
        dve = [i for i in bb.instructions
               if str(i.engine).endswith("DVE")]
        for idx, inst in enumerate(dve):
            if type(inst).__name__ != "InstTensorScalarPtr":
                continue
            for off in (1, 2, 3, 4):   # mt, at, spacer, c
                if idx + off >= len(dve):
                    continue
                tgt = dve[idx + off]
                if type(tgt).__name__ != "InstTensorTensor":
                    continue
                si = tgt.sync_info
                if si is None or not si.on_wait:
                    continue
                waits = [u for u in si.on_wait if not (
                    str(u.sync_type) == "semaphore"
                    and "ge-imm" in str(u.wait_mode)
                    and isinstance(u.ant_name, str)
                    and u.ant_name.startswith("DVE_"))]
                if len(waits) != len(si.on_wait):
                    si.on_wait = waits


_NC_CACHE: dict = {}


def _get_nc(T: int, with_bias: bool) -> bacc.Bacc:
    key = (T, with_bias)
    if key not in _NC_CACHE:
        _NC_CACHE[key] = build_lstm_nc(T, with_bias)
    return _NC_CACHE[key]


def prep_inputs(x, Wx, Wh, b):
    """Host-side prep: pre-scale the g-gate (tanh) columns by 2, transpose
    x to [D, T, B] per core."""
    wx_s = np.array(Wx, dtype=np.float32, copy=True)
    wh_s = np.array(Wh, dtype=np.float32, copy=True)
    b_s = np.array(b, dtype=np.float32, copy=True)
    wx_s[:, 2 * H:3 * H] *= 2.0
    wh_s[:, 2 * H:3 * H] *= 2.0
    b_s[2 * H:3 * H] *= 2.0
    with_bias = bool(np.any(b_s != 0.0))
    x = np.asarray(x, dtype=np.float32)
    wh_f16 = wh_s.astype(np.float16)
    in_maps = []
    for i in range(NCORES):
        m = {
            "xT": np.ascontiguousarray(
                x[i * B_LOC:(i + 1) * B_LOC].transpose(2, 1, 0)),
            "wx": wx_s,
            "wh": wh_f16,
        }
        if with_bias:
            m["bvec"] = b_s.reshape(1, G4)
        in_maps.append(m)
    return in_maps, with_bias


def run(x, Wx, Wh, b, T=None, trace=False):
    T = T if T is not None else x.shape[1]
    in_maps, with_bias = prep_inputs(x, Wx, Wh, b)
    nc = _get_nc(T, with_bias)
    res = bass_utils.run_bass_kernel_spmd(
        nc, in_maps, list(range(NCORES)), trace=trace)
    B = x.shape[0]
    hs = np.empty((B, T, H), dtype=np.float32)
    cs = np.empty((B, T, H), dtype=np.float32)
    for i in range(NCORES):
        hs[i * B_LOC:(i + 1) * B_LOC] = (
            res.results[i]["hsT"].astype(np.float32).transpose(2, 1, 0))
        cs[i * B_LOC:(i + 1) * B_LOC] = (
            res.results[i]["csT"].transpose(2, 1, 0))
    return (hs, cs), res


def kernel(x, Wx, Wh, b):
    (hs, cs), _ = run(x, Wx, Wh, b)
    return hs, cs
